# revision 51
# baseline (speedup 1.0000x reference)
"""Trainium2 Bass kernel: batched multi-head attention (B=2, H=16, S=2048, D=64, fp32).

Full (unsharded) contract: kernel(query, key, value) -> out, all [2, 16, 2048, 64] fp32.

Sharding: the 32 (b, h) pairs are split across 8 NeuronCores, 4 heads per core
(data/head parallel, no communication). Each core runs the same NEFF (SPMD) on
its own 4 heads.

Host-side layout prep (numpy, no FLOPs beyond a bf16 cast of V):
  QT [h, 128, 2048] fp32: Q^T (d on partitions) duplicated into partitions
     64..127 so both PE-array row-halves have the moving operand in place.
  KT [h, 128, 1024] fp32: K^T; partitions 0..63 hold k = 0..1023 (k-tiles 0-7,
     "top"), partitions 64..127 hold k = 1024..2047 (k-tiles 8-15, "bottom").
  VV [h, 128, 16, 65] bf16: V rows permuted so tile t row p = v[t*128 + p],
     with a ones column at index 64 (the PV matmul then emits the softmax
     denominators as OT row 64 for free).

Per-head pipeline on one core (S=2048, D=64, q-chunk 512):
  Per chunk the 16 k-tiles are processed as 6 groups (sizes 3,3,3,3,2,2); a
  group's score tile S^T [128, 512*n] is computed by fp32r matmul with k-tiles
  from opposite halves row-packed in the two 64-row halves of the PE array
  (the d=64 contraction only fills half the array). One ScalarE activation
  exp's the whole group (N up to 1536, fp32 PSUM -> bf16 SBUF, scale folded
  in). PV accumulates OT[65, 512] += (V|1).T @ P^T per k-tile in bf16.
  Drain per chunk: DVE copy OT->SBUF, 4 PE transposes into PSUM, reciprocal
  of the denominator column, per-partition scalar multiply, DMA out.

The engines execute their streams in order; QK runs one group ahead and PV one
group behind the exp stream so ScalarE (the roofline engine here: 16.8M exps
per core at 1 elem/cycle/lane @ 1.2 GHz ~= 109 us floor) never waits.

exp needs no max-subtraction: scores*scale ~ N(0,1) (|s| < ~7), well within
fp32 exp range, and the reference softmax is shift-invariant.
"""

import contextlib
import os
from collections import deque
from contextlib import ExitStack

import numpy as np

B, H, S, D = 2, 16, 2048, 64
BH = B * H
N_CORES = 8
HPC = BH // N_CORES      # heads per core = 4
P = 128
QC = 512                 # q-chunk
NQ = S // QC             # 4
T = S // P               # 16 k-tiles per head
SCALE = 1.0 / float(np.sqrt(D))

# Per-chunk k-tile order (alternating top/bottom halves). The 16 k-tiles of
# a chunk are exp'd in 5 groups whose score tiles strictly alternate between
# two PSUM buffers A (4 banks, <= 2048 cols) and B (3 banks, <= 1536 cols);
# group sizes 4,3,4,3,2 on even chunks and 3,4,3,4,2 on odd chunks keep that
# alternation seamless across chunks (no same-buffer back-to-back, so the
# exp stream never waits on a buffer round-trip).
KSEQ = [0, 8, 1, 9, 2, 10, 3, 11, 4, 12, 5, 13, 6, 14, 7, 15]
STRUCT = os.environ.get("STRUCT", "ab")
if STRUCT == "ab":
    EVEN_SPLITS = [(0, 4), (4, 7), (7, 11), (11, 14), (14, 16)]
    ODD_SPLITS = [(0, 3), (3, 7), (7, 10), (10, 14), (14, 16)]
    SA_PAD = 2048
else:  # uniform 3,3,3,3,2,2 in both buffers
    EVEN_SPLITS = [(0, 3), (3, 6), (6, 9), (9, 12), (12, 14), (14, 16)]
    ODD_SPLITS = EVEN_SPLITS
    SA_PAD = 1536

# PV dtype for P and V ("bf16" | "fp8"). fp8 measures ~4e-2 rel err (beyond
# the 2e-2 gate) -- kept only as an experiment knob.
PV_MODE = os.environ.get("PV_MODE", "bf16")

_RUNNERS: dict = {}


def _vw(pv_mode):
    # fp8 DoubleRow needs the VE row stride (Ko step) 16B-aligned: pad 65->80
    return 80 if pv_mode == "fp8" else D + 1


def build_attention(nc, tc, ctx, qt, kt, vv, o, n_heads, reps=1,
                    parts=frozenset(("exp", "pv", "drain")), pv_mode=PV_MODE):
    from concourse import mybir

    F32 = mybir.dt.float32
    BF16 = mybir.dt.bfloat16
    PVDT = mybir.dt.float8e4 if pv_mode == "fp8" else BF16
    VW = _vw(pv_mode)
    DR = mybir.MatmulPerfMode.DoubleRow
    EXP = mybir.ActivationFunctionType.Exp

    consts = ctx.enter_context(tc.tile_pool(name="consts", bufs=1))
    # preload the exp table set before the main loop needs it
    warm = consts.tile([1, 2], F32)
    nc.vector.memset(warm, 0.0)
    nc.scalar.activation(warm, warm, EXP)
    # ones row used as the outer-product lhsT; lives at partition 64 so its
    # base partition matches the denominator row of ots
    ones_t = consts.tile([P, D], F32)
    nc.vector.memset(ones_t, 1.0)
    ones1 = ones_t[D : D + 1, :]
    # fp8 e4m3 tops out at 240 (inf above): shift scores down by a constant
    # so exp stays in range -- softmax is shift-invariant and the denominator
    # is built from the same shifted values, so the shift cancels exactly.
    exp_bias = 0.0
    if pv_mode == "fp8":
        exp_bias = consts.tile([P, 1], F32)
        nc.vector.memset(exp_bias, -2.5)

    qt_pool = ctx.enter_context(tc.tile_pool(name="qt", bufs=2))
    kt_pool = ctx.enter_context(tc.tile_pool(name="kt", bufs=2))
    ve_pool = ctx.enter_context(tc.tile_pool(name="ve", bufs=2))
    # PSUM: sa 4 banks + sb 3 banks + ot 1 bank = 8 banks exactly.
    sa_pool = ctx.enter_context(tc.tile_pool(name="sa_ps", bufs=1, space="PSUM"))
    sb_pool = ctx.enter_context(tc.tile_pool(name="sb_ps", bufs=1, space="PSUM"))
    ot_pool = ctx.enter_context(tc.tile_pool(name="ot_ps", bufs=1, space="PSUM"))
    pt_pool = ctx.enter_context(tc.tile_pool(name="pt", bufs=3))
    ots_pool = ctx.enter_context(tc.tile_pool(name="ots", bufs=2))
    rc_pool = ctx.enter_context(tc.tile_pool(name="rc", bufs=2))
    stage_pool = ctx.enter_context(tc.tile_pool(name="stage", bufs=2))

    side = deque()
    live: dict = {}

    def pump(n):
        for _ in range(n):
            if not side:
                return
            side.popleft()()

    def load_head(h):
        def f():
            st = {}
            st["QT"] = qt_pool.tile([P, S], BF16, tag="qt", name="QT")
            st["KT"] = kt_pool.tile([P, S // 2], BF16, tag="kt", name="KT")
            st["VE"] = ve_pool.tile([P, T, VW], PVDT, tag="ve", name="VE")
            nc.sync.dma_start(out=st["QT"], in_=qt[h])
            nc.sync.dma_start(out=st["KT"], in_=kt[h])
            nc.sync.dma_start(out=st["VE"], in_=vv[h])
            live[h] = st
        return f

    def emit_qk(step):
        h, qc, slots, which = step
        kts = [KSEQ[s] for s in slots]
        st = live[h]
        if which == "a":
            s_t = sa_pool.tile(
                [P, 512 * len(kts)], F32, padded_shape=[P, SA_PAD], tag="sa",
                name="sa_ps",
            )
        else:
            s_t = sb_pool.tile(
                [P, 512 * len(kts)], F32, padded_shape=[P, 1536], tag="sb",
                name="sb_ps",
            )
        # emit the top/bottom pair back-to-back so the row-packed matmuls
        # overlap in the array; solo (if any) follows
        tops = [x for x in kts if x < 8]
        bots = [x for x in kts if x >= 8]
        order = []
        while tops and bots:
            order.append(tops.pop(0))
            order.append(bots.pop(0))
        order += tops + bots
        for kt_i in order:
            i = kts.index(kt_i)
            half = 0 if kt_i < 8 else 1
            rows = slice(64 * half, 64 * half + 64)
            col = (kt_i % 8) * P
            nc.tensor.matmul(
                s_t[:, i * 512 : (i + 1) * 512],
                st["KT"][rows, col : col + P],
                st["QT"][rows, qc * 512 : (qc + 1) * 512],
                start=True,
                stop=True,
            )
        return s_t

    def emit_exp(s_t, n):
        pt = pt_pool.tile(
            [P, 512 * n], PVDT, padded_shape=[P, 2048], tag="pt", name="pt"
        )
        nc.scalar.activation(pt, s_t, EXP, scale=SCALE, bias=exp_bias)
        return pt

    dev_norm = os.environ.get("DEV_NORM", "0") == "1"
    if dev_norm:
        # DRAM scratch rows for the reciprocal-row roundtrip (DRAM APs allow
        # the zero-step partition broadcast that SBUF APs reject)
        dsc = nc.dram_tensor("dscratch", [2, 1, 512], mybir.dt.float32,
                             kind="Internal").ap()
        dscratch = [dsc[0], dsc[1]]

    def drain(h, qc, OT):
        """Per-chunk drain: copy OT [65, 512] out of PSUM, then either ship
        it raw (host divides + transposes) or normalize on device first.

        The device-normalize path never touches PSUM or TensorE, so the
        next chunk's PV only ever waits on the copy: the denominator row is
        reshaped across 64 partitions by a tiny SBUF->SBUF DMA (so DVE's
        8-cycle-per-element reciprocal runs on 8 elems/lane instead of 512),
        reshaped back, DMA-broadcast across the 64 d-partitions, and
        multiplied into O^T on DVE."""
        ots = ots_pool.tile([D + 1, 512], F32, tag="ots", name="ots")
        nc.vector.tensor_copy(ots, OT[0 : D + 1, :])  # frees OT for next chunk
        if not dev_norm:
            def store():
                nc.sync.dma_start(out=o[h, qc], in_=ots)

            side.append(store)
            return

        box = {}

        def reshape():
            box["rc8"] = rc_pool.tile([D, 8], F32, tag="rc8", name="rc8")
            nc.sync.dma_start(out=box["rc8"], in_=ots[D : D + 1, :])

        def recip():
            nc.vector.reciprocal(box["rc8"], box["rc8"])
            box["rd"] = dscratch[drain.idx % 2]
            drain.idx += 1
            nc.sync.dma_start(out=box["rd"], in_=box["rc8"])

        def bcast():
            box["bc"] = stage_pool.tile([D, 512], F32, tag="bc", name="bc")
            nc.sync.dma_start(
                out=box["bc"], in_=box["rd"].to_broadcast([D, 512])
            )

        def mul():
            box["onrm"] = stage_pool.tile([D, 512], F32, tag="onrm", name="onrm")
            nc.vector.tensor_mul(box["onrm"], ots[0:D, :], box["bc"])

        def store():
            nc.sync.dma_start(out=o[h, qc][0:D, :], in_=box["onrm"])

        side.extend([reshape, recip, bcast, mul, store])

    drain.idx = 0

    ot_state: dict = {}

    def emit_pv(step, pt):
        h, qc, slots, _which = step
        n = len(slots)
        key = (h, qc)
        if key not in ot_state:
            ot_state[key] = [
                ot_pool.tile([VW, 512], F32, tag="ot", name="OT"), 0
            ]
        ent = ot_state[key]
        ve = live[h]["VE"]
        if pv_mode == "fp8" and os.environ.get("PV_DR", "1") == "1":
            # pair-contract adjacent slots with DoubleRow (VE is
            # slot-ordered, so they're adjacent); odd leftover slot normal
            pt_r = pt.rearrange("p (s n) -> p s n", n=512)
            i = 0
            while i < n:
                if i + 1 < n:
                    nc.tensor.matmul(
                        ent[0],
                        ve[:, slots[i] : slots[i] + 2, :],
                        pt_r[:, i : i + 2, :],
                        start=(ent[1] == 0),
                        stop=(ent[1] + 2 == T),
                        perf_mode=DR,
                    )
                    ent[1] += 2
                    i += 2
                else:
                    nc.tensor.matmul(
                        ent[0],
                        ve[:, slots[i], :],
                        pt_r[:, i, :],
                        start=(ent[1] == 0),
                        stop=(ent[1] + 1 == T),
                    )
                    ent[1] += 1
                    i += 1
        else:
            for i in range(n):
                nc.tensor.matmul(
                    ent[0],
                    ve[:, slots[i], 0 : VW if pv_mode == "fp8" else D + 1],
                    pt[:, i * 512 : (i + 1) * 512],
                    start=(ent[1] == 0),
                    stop=(ent[1] + 1 == T),
                )
                ent[1] += 1
        if ent[1] == T:
            if "drain" in parts:
                drain(h, qc, ent[0])
            del ot_state[key]

    steps = []
    gidx = 0
    for h in range(n_heads):
        for qc in range(NQ):
            splits = EVEN_SPLITS if (h * NQ + qc) % 2 == 0 else ODD_SPLITS
            for a, b in splits:
                steps.append((h, qc, list(range(a, b)), "ab"[gidx % 2]))
                gidx += 1
    steps_per_head = NQ * len(EVEN_SPLITS)

    load_head(0)()  # prologue

    rep_ctx = tc.For_i(0, reps, 1) if reps > 1 else contextlib.nullcontext()
    with rep_ctx:
        s_tiles = {0: emit_qk(steps[0])}
        pending = None
        for i, step in enumerate(steps):
            if i % steps_per_head == 0:
                # start loading the next head (head 0 again at the tail, for
                # the next hw-loop rep; harmless overlapped prefetch if reps=1)
                side.append(load_head((step[0] + 1) % n_heads))
            if i + 1 < len(steps):
                s_tiles[i + 1] = emit_qk(steps[i + 1])
            if "exp" in parts:
                pt = emit_exp(s_tiles.pop(i), len(step[2]))
                if "pv" in parts:
                    if pending is not None:
                        emit_pv(*pending)
                    pending = (step, pt)
            else:
                s_tiles.pop(i)
            pump(2 if len(side) <= 8 else 3)
        if pending is not None:
            emit_pv(*pending)
        while side:
            side.popleft()()


def build_nc(n_heads=HPC, reps=1, name="attn",
             parts=frozenset(("exp", "pv", "drain")), pv_mode=PV_MODE):
    """Build + compile the per-core Bass program."""
    import concourse.tile as tile
    from concourse import bacc, mybir

    nc = bacc.Bacc(
        "TRN2",
        target_bir_lowering=False,
        debug=False,
        num_devices=N_CORES,
        name=name,
    )
    qtd = nc.dram_tensor(
        "qt", [n_heads, P, S], mybir.dt.bfloat16, kind="ExternalInput"
    ).ap()
    ktd = nc.dram_tensor(
        "kt", [n_heads, P, S // 2], mybir.dt.bfloat16, kind="ExternalInput"
    ).ap()
    vvd = nc.dram_tensor(
        "vv",
        [n_heads, P, T, _vw(pv_mode)],
        mybir.dt.float8e4 if pv_mode == "fp8" else mybir.dt.bfloat16,
        kind="ExternalInput",
    ).ap()
    od = nc.dram_tensor(
        "o", [n_heads, NQ, D + 1, QC], mybir.dt.float32, kind="ExternalOutput"
    ).ap()

    with tile.TileContext(nc) as tc:
        with ExitStack() as ctx:
            build_attention(
                nc, tc, ctx, qtd, ktd, vvd, od, n_heads, reps, parts, pv_mode
            )
    nc.compile()
    return nc


class PjrtRunner:
    """Build-once / run-many PJRT executor for a compiled Bass program.

    Mirrors concourse.bass2jax.run_bass_via_pjrt, but holds onto the jitted
    callable so repeat invocations don't re-trace (and re-run neuronxcc).
    """

    def __init__(self, nc, n_cores=N_CORES):
        import jax
        from jax.experimental.shard_map import shard_map
        from jax.sharding import Mesh, PartitionSpec

        import concourse.mybir as mybir
        from concourse.bass2jax import _bass_exec_p, install_neuronx_cc_hook

        install_neuronx_cc_hook()
        self.nc = nc
        self.n_cores = n_cores

        in_names, out_names, out_avals, zero_outs = [], [], [], []
        partition_name = (
            nc.partition_id_tensor.name if nc.partition_id_tensor else None
        )
        for alloc in nc.m.functions[0].allocations:
            if not isinstance(alloc, mybir.MemoryLocationSet):
                continue
            nm = alloc.memorylocations[0].name
            if alloc.kind == "ExternalInput":
                if nm != partition_name:
                    in_names.append(nm)
            elif alloc.kind == "ExternalOutput":
                shape = tuple(alloc.tensor_shape)
                dtype = mybir.dt.np(alloc.dtype)
                out_names.append(nm)
                out_avals.append(jax.core.ShapedArray(shape, dtype))
                zero_outs.append(np.zeros(shape, dtype))
        self.in_names = list(in_names)
        self.out_names = out_names
        self.out_avals = out_avals
        self.zero_outs = zero_outs
        n_params = len(in_names)
        n_outs = len(out_avals)
        all_in_names = list(in_names) + list(out_names)
        if partition_name is not None:
            all_in_names.append(partition_name)

        def _body(*args):
            operands = list(args)
            if partition_name is not None:
                from concourse.bass2jax import partition_id_tensor

                operands.append(partition_id_tensor())
            outs = _bass_exec_p.bind(
                *operands,
                out_avals=tuple(out_avals),
                in_names=tuple(all_in_names),
                out_names=tuple(out_names),
                lowering_input_output_aliases=(),
                sim_require_finite=True,
                sim_require_nnan=True,
                nc=nc,
            )
            return tuple(outs)

        donate = tuple(range(n_params, n_params + n_outs))
        devices = jax.devices()[:n_cores]
        assert len(devices) == n_cores
        self.mesh = Mesh(np.asarray(devices), ("core",))
        in_specs = (PartitionSpec("core"),) * (n_params + n_outs)
        out_specs = (PartitionSpec("core"),) * n_outs
        self.fn = jax.jit(
            shard_map(
                _body,
                mesh=self.mesh,
                in_specs=in_specs,
                out_specs=out_specs,
                check_rep=False,
            ),
            donate_argnums=donate,
            keep_unused=True,
        )

    def _concat_inputs(self, in_maps):
        concat = [
            np.concatenate([np.asarray(m[nm]) for m in in_maps], axis=0)
            for nm in self.in_names
        ]
        zeros = [
            np.zeros((self.n_cores * z.shape[0], *z.shape[1:]), z.dtype)
            for z in self.zero_outs
        ]
        return concat, zeros

    def run(self, in_maps):
        concat, zeros = self._concat_inputs(in_maps)
        out_arrs = self.fn(*concat, *zeros)
        return [
            {
                nm: np.asarray(out_arrs[i]).reshape(
                    self.n_cores, *self.out_avals[i].shape
                )[c]
                for i, nm in enumerate(self.out_names)
            }
            for c in range(self.n_cores)
        ]

    def time_calls(self, in_maps, iters=5):
        """Wall-clock dispatches with all buffers device-resident.

        Per-call time = axon dispatch RTT + NEFF execution; differencing two
        rep-count variants cancels the RTT."""
        import time as _time

        import jax
        from jax.sharding import NamedSharding, PartitionSpec

        concat, zeros = self._concat_inputs(in_maps)
        sh = NamedSharding(self.mesh, PartitionSpec("core"))
        dev_in = [jax.device_put(c, sh) for c in concat]
        zs_sets = [[jax.device_put(z, sh) for z in zeros] for _ in range(iters)]
        for s in zs_sets:
            for a in s:
                a.block_until_ready()
        # warmup (compile)
        out = self.fn(*dev_in, *[jax.device_put(z, sh) for z in zeros])
        for a in out:
            a.block_until_ready()
        times = []
        for i in range(iters):
            t0 = _time.perf_counter()
            out = self.fn(*dev_in, *zs_sets[i])
            for a in out:
                a.block_until_ready()
            times.append(_time.perf_counter() - t0)
        return times


def time_interleaved(ra, rb, reps_a, reps_b, in_maps, iters=12):
    """Per-rep HW time via interleaved (reps_b - reps_a) wall differencing.

    Each iteration times a reps_a call then a reps_b call back-to-back so
    dispatch-RTT drift cancels within the pair; returns per-iter estimates.
    """
    import time as _time

    import jax
    from jax.sharding import NamedSharding, PartitionSpec

    def prep(r):
        concat, zeros = r._concat_inputs(in_maps)
        sh = NamedSharding(r.mesh, PartitionSpec("core"))
        dev_in = [jax.device_put(c, sh) for c in concat]
        zs = [[jax.device_put(z, sh) for z in zeros] for _ in range(iters + 1)]
        for s in zs:
            for a in s:
                a.block_until_ready()
        return dev_in, zs

    in_a, zs_a = prep(ra)
    in_b, zs_b = prep(rb)
    # warmup/compile both
    for r, di, z in ((ra, in_a, zs_a), (rb, in_b, zs_b)):
        out = r.fn(*di, *z[iters])
        for a in out:
            a.block_until_ready()
    ests = []
    for i in range(iters):
        t0 = _time.perf_counter()
        out = ra.fn(*in_a, *zs_a[i])
        for a in out:
            a.block_until_ready()
        t1 = _time.perf_counter()
        out = rb.fn(*in_b, *zs_b[i])
        for a in out:
            a.block_until_ready()
        t2 = _time.perf_counter()
        ests.append(((t2 - t1) - (t1 - t0)) / (reps_b - reps_a))
    return ests


def _get_runner(reps=1):
    key = ("runner", reps)
    if key not in _RUNNERS:
        nc = build_nc(reps=reps, name=f"attn_r{reps}")
        _RUNNERS[key] = PjrtRunner(nc)
    return _RUNNERS[key]


def _prepare(query, key, value):
    """Host-side layout prep; returns per-core {qt, kt, vv} input maps."""
    import ml_dtypes

    q = np.ascontiguousarray(np.asarray(query), dtype=np.float32).reshape(BH, S, D)
    k = np.ascontiguousarray(np.asarray(key), dtype=np.float32).reshape(BH, S, D)
    v = np.ascontiguousarray(np.asarray(value), dtype=np.float32).reshape(BH, S, D)

    qT = q.transpose(0, 2, 1)                                   # [BH, 64, S]
    qt2 = np.concatenate([qT, qT], axis=1).astype(ml_dtypes.bfloat16)
    kT = k.transpose(0, 2, 1)
    kt2 = np.concatenate(
        [kT[:, :, : S // 2], kT[:, :, S // 2 :]], axis=1
    ).astype(ml_dtypes.bfloat16)
    # V in slot (KSEQ) order with a ones column; fp8 pads the row to 80 so
    # DoubleRow's Ko step is 16B-aligned
    vw = _vw(PV_MODE)
    pvdt = ml_dtypes.float8_e4m3 if PV_MODE == "fp8" else ml_dtypes.bfloat16
    vp = v.reshape(BH, T, P, D)[:, KSEQ].transpose(0, 2, 1, 3)  # [BH, 128, T, 64]
    vv = np.zeros((BH, P, T, vw), dtype=pvdt)
    vv[:, :, :, :D] = vp.astype(pvdt)
    vv[:, :, :, D] = 1.0
    in_maps = []
    for c in range(N_CORES):
        hs = slice(c * HPC, (c + 1) * HPC)
        in_maps.append({"qt": qt2[hs], "kt": kt2[hs], "vv": vv[hs]})
    return in_maps


def kernel(query, key, value):
    os.environ.setdefault("JAX_PLATFORMS", "")
    in_maps = _prepare(query, key, value)
    runner = _get_runner(reps=1)
    results = runner.run(in_maps)
    # device ships O^T per (head, chunk): [HPC, NQ, 65, 512]. With DEV_NORM
    # the device already divided by the softmax denominators; otherwise row
    # 64 carries them and the host divides during output assembly.
    out = np.concatenate([results[c]["o"] for c in range(N_CORES)], axis=0)
    if os.environ.get("DEV_NORM", "0") == "1":
        out = out[:, :, :D, :]
    else:
        out = out[:, :, :D, :] / out[:, :, D : D + 1, :]
    out = out.transpose(0, 1, 3, 2).reshape(BH, S, D)
    return out.reshape(B, H, S, D).astype(np.float32)


# revision 52
# speedup vs baseline: 1.0522x; 1.0522x over previous
"""Trainium2 Bass kernel: batched multi-head attention (B=2, H=16, S=2048, D=64, fp32).

Full (unsharded) contract: kernel(query, key, value) -> out, all [2, 16, 2048, 64] fp32.

Sharding: the 32 (b, h) pairs are split across 8 NeuronCores, 4 heads per core
(data/head parallel, no communication). Each core runs the same NEFF (SPMD) on
its own 4 heads.

Host-side layout prep (numpy; bf16 casts of Q, K, V):
  QT [h, 128, 2048] bf16: Q^T (d on partitions) duplicated into partitions
     64..127 so both PE-array row-halves have the moving operand in place.
     (bf16 weights also enable separate, overlappable LDWEIGHTS + FWL; with
     fp32/fp32r the weight load is fused into the matmul and serializes.)
  KT [h, 128, 1024] bf16: K^T; partitions 0..63 hold k = 0..1023 (k-tiles 0-7,
     "top"), partitions 64..127 hold k = 1024..2047 (k-tiles 8-15, "bottom").
  VV [h, 128, 16, 65] bf16: V k-tiles in KSEQ slot order, row p of slot j =
     v[KSEQ[j]*128 + p], with a ones column at index 64 (the PV matmul then
     emits the softmax denominators as OT row 64 for free).

Per-head pipeline on one core (S=2048, D=64, q-chunk 512):
  Each chunk's 16 k-tiles are exp'd in 5 groups whose fp32 score tiles
  strictly alternate between PSUM buffers A (4 banks, <= 2048 cols) and B
  (3 banks, <= 1536 cols); sizes run 4,3,4,3,2 / 3,4,3,4,2 on alternating
  chunks so the alternation is seamless and the exp stream never waits on a
  buffer round-trip. PSUM: A(4) + B(3) + OT(1) = all 8 banks.
  QK: bf16 matmuls, k-tiles from opposite halves row-packed concurrently in
  the two 64-row halves of the PE array (the d=64 contraction only fills
  half the array). One ScalarE activation exp's each group (fp32 PSUM ->
  bf16 SBUF, 1/sqrt(d) folded into the activation's scale). PV accumulates
  OT[65, 512] += (V|1).T @ P^T per k-tile in bf16 (row 64 = denominators).
  Drain per chunk: DVE copies OT out of PSUM (the only thing the next
  chunk's PV waits on) and DMA ships O^T + denominator row; the host divides
  by the denominators and undoes the O^T layout during output assembly.

The engines execute their streams in order; QK runs one group ahead and PV one
group behind the exp stream so ScalarE (the roofline engine here: 16.8M exps
per core at 1 elem/cycle/lane @ 1.2 GHz ~= 109 us floor; ~17% per-instruction
overhead at these lengths) stays saturated.

exp needs no max-subtraction: scores*scale ~ N(0,1) (|s| < ~7), well within
fp32 exp range, and the reference softmax is shift-invariant.
"""

import contextlib
import os
from collections import deque
from contextlib import ExitStack

import numpy as np

B, H, S, D = 2, 16, 2048, 64
BH = B * H
N_CORES = 8
HPC = BH // N_CORES      # heads per core = 4
P = 128
QC = 512                 # q-chunk
NQ = S // QC             # 4
T = S // P               # 16 k-tiles per head
SCALE = 1.0 / float(np.sqrt(D))

# Per-chunk k-tile order (alternating top/bottom halves). The 16 k-tiles of
# a chunk are exp'd in 5 groups whose score tiles strictly alternate between
# two PSUM buffers A (4 banks, <= 2048 cols) and B (3 banks, <= 1536 cols);
# group sizes 4,3,4,3,2 on even chunks and 3,4,3,4,2 on odd chunks keep that
# alternation seamless across chunks (no same-buffer back-to-back, so the
# exp stream never waits on a buffer round-trip).
KSEQ = [0, 8, 1, 9, 2, 10, 3, 11, 4, 12, 5, 13, 6, 14, 7, 15]
STRUCT = os.environ.get("STRUCT", "ab")
if STRUCT == "ab":
    EVEN_SPLITS = [(0, 4), (4, 7), (7, 11), (11, 14), (14, 16)]
    ODD_SPLITS = [(0, 3), (3, 7), (7, 10), (10, 14), (14, 16)]
    SA_PAD = 2048
else:  # uniform 3,3,3,3,2,2 in both buffers
    EVEN_SPLITS = [(0, 3), (3, 6), (6, 9), (9, 12), (12, 14), (14, 16)]
    ODD_SPLITS = EVEN_SPLITS
    SA_PAD = 1536

# PV dtype for P and V ("bf16" | "fp8"). fp8 measures ~4e-2 rel err (beyond
# the 2e-2 gate) -- kept only as an experiment knob.
PV_MODE = os.environ.get("PV_MODE", "bf16")

_RUNNERS: dict = {}


def _vw(pv_mode):
    # fp8 DoubleRow needs the VE row stride (Ko step) 16B-aligned: pad 65->80
    return 80 if pv_mode == "fp8" else D + 1


def build_attention(nc, tc, ctx, qt, kt, vv, o, n_heads, reps=1,
                    parts=frozenset(("exp", "pv", "drain")), pv_mode=PV_MODE):
    from concourse import mybir

    F32 = mybir.dt.float32
    BF16 = mybir.dt.bfloat16
    PVDT = mybir.dt.float8e4 if pv_mode == "fp8" else BF16
    VW = _vw(pv_mode)
    DR = mybir.MatmulPerfMode.DoubleRow
    EXP = mybir.ActivationFunctionType.Exp

    consts = ctx.enter_context(tc.tile_pool(name="consts", bufs=1))
    # preload the exp table set before the main loop needs it
    warm = consts.tile([1, 2], F32)
    nc.vector.memset(warm, 0.0)
    nc.scalar.activation(warm, warm, EXP)
    # ones row used as the outer-product lhsT; lives at partition 64 so its
    # base partition matches the denominator row of ots
    ones_t = consts.tile([P, D], F32)
    nc.vector.memset(ones_t, 1.0)
    ones1 = ones_t[D : D + 1, :]
    # fp8 e4m3 tops out at 240 (inf above): shift scores down by a constant
    # so exp stays in range -- softmax is shift-invariant and the denominator
    # is built from the same shifted values, so the shift cancels exactly.
    exp_bias = 0.0
    if pv_mode == "fp8":
        exp_bias = consts.tile([P, 1], F32)
        nc.vector.memset(exp_bias, -2.5)

    qt_pool = ctx.enter_context(tc.tile_pool(name="qt", bufs=2))
    kt_pool = ctx.enter_context(tc.tile_pool(name="kt", bufs=2))
    ve_pool = ctx.enter_context(tc.tile_pool(name="ve", bufs=2))
    # PSUM: sa 4 banks + sb 3 banks + ot 1 bank = 8 banks exactly.
    sa_pool = ctx.enter_context(tc.tile_pool(name="sa_ps", bufs=1, space="PSUM"))
    sb_pool = ctx.enter_context(tc.tile_pool(name="sb_ps", bufs=1, space="PSUM"))
    ot_pool = ctx.enter_context(tc.tile_pool(name="ot_ps", bufs=1, space="PSUM"))
    pt_pool = ctx.enter_context(tc.tile_pool(name="pt", bufs=3))
    ots_pool = ctx.enter_context(tc.tile_pool(name="ots", bufs=2))
    rc_pool = ctx.enter_context(tc.tile_pool(name="rc", bufs=2))
    stage_pool = ctx.enter_context(tc.tile_pool(name="stage", bufs=2))

    side = deque()
    live: dict = {}

    def pump(n):
        for _ in range(n):
            if not side:
                return
            side.popleft()()

    def load_head(h):
        def f():
            st = {}
            st["QT"] = qt_pool.tile([P, S], BF16, tag="qt", name="QT")
            st["KT"] = kt_pool.tile([P, S // 2], BF16, tag="kt", name="KT")
            st["VE"] = ve_pool.tile([P, T, VW], PVDT, tag="ve", name="VE")
            nc.sync.dma_start(out=st["QT"], in_=qt[h])
            nc.sync.dma_start(out=st["KT"], in_=kt[h])
            nc.sync.dma_start(out=st["VE"], in_=vv[h])
            live[h] = st
        return f

    def emit_qk(step):
        h, qc, slots, which = step
        kts = [KSEQ[s] for s in slots]
        st = live[h]
        if which == "a":
            s_t = sa_pool.tile(
                [P, 512 * len(kts)], F32, padded_shape=[P, SA_PAD], tag="sa",
                name="sa_ps",
            )
        else:
            s_t = sb_pool.tile(
                [P, 512 * len(kts)], F32, padded_shape=[P, 1536], tag="sb",
                name="sb_ps",
            )
        # emit the top/bottom pair back-to-back so the row-packed matmuls
        # overlap in the array; solo (if any) follows
        tops = [x for x in kts if x < 8]
        bots = [x for x in kts if x >= 8]
        order = []
        while tops and bots:
            order.append(tops.pop(0))
            order.append(bots.pop(0))
        order += tops + bots
        for kt_i in order:
            i = kts.index(kt_i)
            half = 0 if kt_i < 8 else 1
            rows = slice(64 * half, 64 * half + 64)
            col = (kt_i % 8) * P
            nc.tensor.matmul(
                s_t[:, i * 512 : (i + 1) * 512],
                st["KT"][rows, col : col + P],
                st["QT"][rows, qc * 512 : (qc + 1) * 512],
                start=True,
                stop=True,
            )
        return s_t

    def emit_exp(s_t, n):
        pt = pt_pool.tile(
            [P, 512 * n], PVDT, padded_shape=[P, 2048], tag="pt", name="pt"
        )
        nc.scalar.activation(pt, s_t, EXP, scale=SCALE, bias=exp_bias)
        return pt

    dev_norm = os.environ.get("DEV_NORM", "0") == "1"
    if dev_norm:
        # DRAM scratch rows for the reciprocal-row roundtrip (DRAM APs allow
        # the zero-step partition broadcast that SBUF APs reject)
        dsc = nc.dram_tensor("dscratch", [2, 1, 512], mybir.dt.float32,
                             kind="Internal").ap()
        dscratch = [dsc[0], dsc[1]]

    def drain(h, qc, OT):
        """Per-chunk drain: copy OT [65, 512] out of PSUM, then either ship
        it raw (host divides + transposes) or normalize on device first.

        The device-normalize path never touches PSUM or TensorE, so the
        next chunk's PV only ever waits on the copy: the denominator row is
        reshaped across 64 partitions by a tiny SBUF->SBUF DMA (so DVE's
        8-cycle-per-element reciprocal runs on 8 elems/lane instead of 512),
        reshaped back, DMA-broadcast across the 64 d-partitions, and
        multiplied into O^T on DVE."""
        ots = ots_pool.tile([D + 1, 512], F32, tag="ots", name="ots")
        nc.vector.tensor_copy(ots, OT[0 : D + 1, :])  # frees OT for next chunk
        if not dev_norm:
            def store():
                nc.sync.dma_start(out=o[h, qc], in_=ots)

            side.append(store)
            return

        box = {}

        def reshape():
            box["rc8"] = rc_pool.tile([D, 8], F32, tag="rc8", name="rc8")
            nc.sync.dma_start(out=box["rc8"], in_=ots[D : D + 1, :])

        def recip():
            nc.vector.reciprocal(box["rc8"], box["rc8"])
            box["rd"] = dscratch[drain.idx % 2]
            drain.idx += 1
            nc.sync.dma_start(out=box["rd"], in_=box["rc8"])

        def bcast():
            box["bc"] = stage_pool.tile([D, 512], F32, tag="bc", name="bc")
            nc.sync.dma_start(
                out=box["bc"], in_=box["rd"].to_broadcast([D, 512])
            )

        def mul():
            box["onrm"] = stage_pool.tile([D, 512], F32, tag="onrm", name="onrm")
            nc.vector.tensor_mul(box["onrm"], ots[0:D, :], box["bc"])

        def store():
            nc.sync.dma_start(out=o[h, qc][0:D, :], in_=box["onrm"])

        side.extend([reshape, recip, bcast, mul, store])

    drain.idx = 0

    ot_state: dict = {}

    def emit_pv(step, pt):
        h, qc, slots, _which = step
        n = len(slots)
        key = (h, qc)
        if key not in ot_state:
            ot_state[key] = [
                ot_pool.tile([VW, 512], F32, tag="ot", name="OT"), 0
            ]
        ent = ot_state[key]
        ve = live[h]["VE"]
        if pv_mode == "fp8" and os.environ.get("PV_DR", "1") == "1":
            # pair-contract adjacent slots with DoubleRow (VE is
            # slot-ordered, so they're adjacent); odd leftover slot normal
            pt_r = pt.rearrange("p (s n) -> p s n", n=512)
            i = 0
            while i < n:
                if i + 1 < n:
                    nc.tensor.matmul(
                        ent[0],
                        ve[:, slots[i] : slots[i] + 2, :],
                        pt_r[:, i : i + 2, :],
                        start=(ent[1] == 0),
                        stop=(ent[1] + 2 == T),
                        perf_mode=DR,
                    )
                    ent[1] += 2
                    i += 2
                else:
                    nc.tensor.matmul(
                        ent[0],
                        ve[:, slots[i], :],
                        pt_r[:, i, :],
                        start=(ent[1] == 0),
                        stop=(ent[1] + 1 == T),
                    )
                    ent[1] += 1
                    i += 1
        else:
            for i in range(n):
                nc.tensor.matmul(
                    ent[0],
                    ve[:, slots[i], 0 : VW if pv_mode == "fp8" else D + 1],
                    pt[:, i * 512 : (i + 1) * 512],
                    start=(ent[1] == 0),
                    stop=(ent[1] + 1 == T),
                )
                ent[1] += 1
        if ent[1] == T:
            if "drain" in parts:
                drain(h, qc, ent[0])
            del ot_state[key]

    steps = []
    gidx = 0
    for h in range(n_heads):
        for qc in range(NQ):
            splits = EVEN_SPLITS if (h * NQ + qc) % 2 == 0 else ODD_SPLITS
            for a, b in splits:
                steps.append((h, qc, list(range(a, b)), "ab"[gidx % 2]))
                gidx += 1
    steps_per_head = NQ * len(EVEN_SPLITS)

    load_head(0)()  # prologue

    rep_ctx = tc.For_i(0, reps, 1) if reps > 1 else contextlib.nullcontext()
    with rep_ctx:
        s_tiles = {0: emit_qk(steps[0])}
        pending = None
        for i, step in enumerate(steps):
            if i % steps_per_head == 0:
                # start loading the next head (head 0 again at the tail, for
                # the next hw-loop rep; harmless overlapped prefetch if reps=1)
                side.append(load_head((step[0] + 1) % n_heads))
            if i + 1 < len(steps):
                s_tiles[i + 1] = emit_qk(steps[i + 1])
            if "exp" in parts:
                pt = emit_exp(s_tiles.pop(i), len(step[2]))
                if "pv" in parts:
                    if pending is not None:
                        emit_pv(*pending)
                    pending = (step, pt)
            else:
                s_tiles.pop(i)
            pump(2 if len(side) <= 8 else 3)
        if pending is not None:
            emit_pv(*pending)
        while side:
            side.popleft()()


def build_nc(n_heads=HPC, reps=1, name="attn",
             parts=frozenset(("exp", "pv", "drain")), pv_mode=PV_MODE):
    """Build + compile the per-core Bass program."""
    import concourse.tile as tile
    from concourse import bacc, mybir

    nc = bacc.Bacc(
        "TRN2",
        target_bir_lowering=False,
        debug=False,
        num_devices=N_CORES,
        name=name,
    )
    qtd = nc.dram_tensor(
        "qt", [n_heads, P, S], mybir.dt.bfloat16, kind="ExternalInput"
    ).ap()
    ktd = nc.dram_tensor(
        "kt", [n_heads, P, S // 2], mybir.dt.bfloat16, kind="ExternalInput"
    ).ap()
    vvd = nc.dram_tensor(
        "vv",
        [n_heads, P, T, _vw(pv_mode)],
        mybir.dt.float8e4 if pv_mode == "fp8" else mybir.dt.bfloat16,
        kind="ExternalInput",
    ).ap()
    od = nc.dram_tensor(
        "o", [n_heads, NQ, D + 1, QC], mybir.dt.float32, kind="ExternalOutput"
    ).ap()

    with tile.TileContext(nc) as tc:
        with ExitStack() as ctx:
            build_attention(
                nc, tc, ctx, qtd, ktd, vvd, od, n_heads, reps, parts, pv_mode
            )
    nc.compile()
    return nc


class PjrtRunner:
    """Build-once / run-many PJRT executor for a compiled Bass program.

    Mirrors concourse.bass2jax.run_bass_via_pjrt, but holds onto the jitted
    callable so repeat invocations don't re-trace (and re-run neuronxcc).
    """

    def __init__(self, nc, n_cores=N_CORES):
        import jax
        from jax.experimental.shard_map import shard_map
        from jax.sharding import Mesh, PartitionSpec

        import concourse.mybir as mybir
        from concourse.bass2jax import _bass_exec_p, install_neuronx_cc_hook

        install_neuronx_cc_hook()
        self.nc = nc
        self.n_cores = n_cores

        in_names, out_names, out_avals, zero_outs = [], [], [], []
        partition_name = (
            nc.partition_id_tensor.name if nc.partition_id_tensor else None
        )
        for alloc in nc.m.functions[0].allocations:
            if not isinstance(alloc, mybir.MemoryLocationSet):
                continue
            nm = alloc.memorylocations[0].name
            if alloc.kind == "ExternalInput":
                if nm != partition_name:
                    in_names.append(nm)
            elif alloc.kind == "ExternalOutput":
                shape = tuple(alloc.tensor_shape)
                dtype = mybir.dt.np(alloc.dtype)
                out_names.append(nm)
                out_avals.append(jax.core.ShapedArray(shape, dtype))
                zero_outs.append(np.zeros(shape, dtype))
        self.in_names = list(in_names)
        self.out_names = out_names
        self.out_avals = out_avals
        self.zero_outs = zero_outs
        n_params = len(in_names)
        n_outs = len(out_avals)
        all_in_names = list(in_names) + list(out_names)
        if partition_name is not None:
            all_in_names.append(partition_name)

        def _body(*args):
            operands = list(args)
            if partition_name is not None:
                from concourse.bass2jax import partition_id_tensor

                operands.append(partition_id_tensor())
            outs = _bass_exec_p.bind(
                *operands,
                out_avals=tuple(out_avals),
                in_names=tuple(all_in_names),
                out_names=tuple(out_names),
                lowering_input_output_aliases=(),
                sim_require_finite=True,
                sim_require_nnan=True,
                nc=nc,
            )
            return tuple(outs)

        donate = tuple(range(n_params, n_params + n_outs))
        devices = jax.devices()[:n_cores]
        assert len(devices) == n_cores
        self.mesh = Mesh(np.asarray(devices), ("core",))
        in_specs = (PartitionSpec("core"),) * (n_params + n_outs)
        out_specs = (PartitionSpec("core"),) * n_outs
        self.fn = jax.jit(
            shard_map(
                _body,
                mesh=self.mesh,
                in_specs=in_specs,
                out_specs=out_specs,
                check_rep=False,
            ),
            donate_argnums=donate,
            keep_unused=True,
        )

    def _concat_inputs(self, in_maps):
        concat = [
            np.concatenate([np.asarray(m[nm]) for m in in_maps], axis=0)
            for nm in self.in_names
        ]
        zeros = [
            np.zeros((self.n_cores * z.shape[0], *z.shape[1:]), z.dtype)
            for z in self.zero_outs
        ]
        return concat, zeros

    def run(self, in_maps):
        concat, zeros = self._concat_inputs(in_maps)
        out_arrs = self.fn(*concat, *zeros)
        return [
            {
                nm: np.asarray(out_arrs[i]).reshape(
                    self.n_cores, *self.out_avals[i].shape
                )[c]
                for i, nm in enumerate(self.out_names)
            }
            for c in range(self.n_cores)
        ]

    def time_calls(self, in_maps, iters=5):
        """Wall-clock dispatches with all buffers device-resident.

        Per-call time = axon dispatch RTT + NEFF execution; differencing two
        rep-count variants cancels the RTT."""
        import time as _time

        import jax
        from jax.sharding import NamedSharding, PartitionSpec

        concat, zeros = self._concat_inputs(in_maps)
        sh = NamedSharding(self.mesh, PartitionSpec("core"))
        dev_in = [jax.device_put(c, sh) for c in concat]
        zs_sets = [[jax.device_put(z, sh) for z in zeros] for _ in range(iters)]
        for s in zs_sets:
            for a in s:
                a.block_until_ready()
        # warmup (compile)
        out = self.fn(*dev_in, *[jax.device_put(z, sh) for z in zeros])
        for a in out:
            a.block_until_ready()
        times = []
        for i in range(iters):
            t0 = _time.perf_counter()
            out = self.fn(*dev_in, *zs_sets[i])
            for a in out:
                a.block_until_ready()
            times.append(_time.perf_counter() - t0)
        return times


def time_interleaved(ra, rb, reps_a, reps_b, in_maps, iters=12):
    """Per-rep HW time via interleaved (reps_b - reps_a) wall differencing.

    Each iteration times a reps_a call then a reps_b call back-to-back so
    dispatch-RTT drift cancels within the pair; returns per-iter estimates.
    """
    import time as _time

    import jax
    from jax.sharding import NamedSharding, PartitionSpec

    def prep(r):
        concat, zeros = r._concat_inputs(in_maps)
        sh = NamedSharding(r.mesh, PartitionSpec("core"))
        dev_in = [jax.device_put(c, sh) for c in concat]
        zs = [[jax.device_put(z, sh) for z in zeros] for _ in range(iters + 1)]
        for s in zs:
            for a in s:
                a.block_until_ready()
        return dev_in, zs

    in_a, zs_a = prep(ra)
    in_b, zs_b = prep(rb)
    # warmup/compile both
    for r, di, z in ((ra, in_a, zs_a), (rb, in_b, zs_b)):
        out = r.fn(*di, *z[iters])
        for a in out:
            a.block_until_ready()
    ests = []
    for i in range(iters):
        t0 = _time.perf_counter()
        out = ra.fn(*in_a, *zs_a[i])
        for a in out:
            a.block_until_ready()
        t1 = _time.perf_counter()
        out = rb.fn(*in_b, *zs_b[i])
        for a in out:
            a.block_until_ready()
        t2 = _time.perf_counter()
        ests.append(((t2 - t1) - (t1 - t0)) / (reps_b - reps_a))
    return ests


def _get_runner(reps=1):
    key = ("runner", reps)
    if key not in _RUNNERS:
        nc = build_nc(reps=reps, name=f"attn_r{reps}")
        _RUNNERS[key] = PjrtRunner(nc)
    return _RUNNERS[key]


def _prepare(query, key, value):
    """Host-side layout prep; returns per-core {qt, kt, vv} input maps."""
    import ml_dtypes

    q = np.ascontiguousarray(np.asarray(query), dtype=np.float32).reshape(BH, S, D)
    k = np.ascontiguousarray(np.asarray(key), dtype=np.float32).reshape(BH, S, D)
    v = np.ascontiguousarray(np.asarray(value), dtype=np.float32).reshape(BH, S, D)

    qT = q.transpose(0, 2, 1)                                   # [BH, 64, S]
    qt2 = np.concatenate([qT, qT], axis=1).astype(ml_dtypes.bfloat16)
    kT = k.transpose(0, 2, 1)
    kt2 = np.concatenate(
        [kT[:, :, : S // 2], kT[:, :, S // 2 :]], axis=1
    ).astype(ml_dtypes.bfloat16)
    # V in slot (KSEQ) order with a ones column; fp8 pads the row to 80 so
    # DoubleRow's Ko step is 16B-aligned
    vw = _vw(PV_MODE)
    pvdt = ml_dtypes.float8_e4m3 if PV_MODE == "fp8" else ml_dtypes.bfloat16
    vp = v.reshape(BH, T, P, D)[:, KSEQ].transpose(0, 2, 1, 3)  # [BH, 128, T, 64]
    vv = np.zeros((BH, P, T, vw), dtype=pvdt)
    vv[:, :, :, :D] = vp.astype(pvdt)
    vv[:, :, :, D] = 1.0
    in_maps = []
    for c in range(N_CORES):
        hs = slice(c * HPC, (c + 1) * HPC)
        in_maps.append({"qt": qt2[hs], "kt": kt2[hs], "vv": vv[hs]})
    return in_maps


def kernel(query, key, value):
    os.environ.setdefault("JAX_PLATFORMS", "")
    in_maps = _prepare(query, key, value)
    runner = _get_runner(reps=1)
    results = runner.run(in_maps)
    # device ships O^T per (head, chunk): [HPC, NQ, 65, 512]. With DEV_NORM
    # the device already divided by the softmax denominators; otherwise row
    # 64 carries them and the host divides during output assembly.
    out = np.concatenate([results[c]["o"] for c in range(N_CORES)], axis=0)
    if os.environ.get("DEV_NORM", "0") == "1":
        out = out[:, :, :D, :]
    else:
        out = out[:, :, :D, :] / out[:, :, D : D + 1, :]
    out = out.transpose(0, 1, 3, 2).reshape(BH, S, D)
    return out.reshape(B, H, S, D).astype(np.float32)
